# revision 23
# baseline (speedup 1.0000x reference)
"""Distributed attention kernel for 8 TRN2 NeuronCores.

Sharding: core c -> (batch b = c//2, head-half hh = c%2).  Each core computes
LN(x_b) for all 2048 rows, q for its 8 heads, k/v over the first KEYSC-1
rows of a HOST-PERMUTED x (visible rows first, masked rows after; the output
is un-permuted on the host), l2norm cosine attention with the null k/v
appended at slot KEYSC-1, and a partial out @ wo[head-slice].  Host sums the
two partial outputs per batch.

Performance notes (779us -> 424us -> this):
  - l2norm(s*row) == l2norm(row): the LN rstd never reaches q/k.  x is
    projected RAW; the mean subtraction is a rank-1 correction matmul
    (stationary = host-precomputed -colsum(gamma*W) row, moving = the mean
    row) accumulated into each projection PSUM tile.  Only 128 MACs active:
    dense ones[128,128] broadcast stationaries trip a chip-wide ~2.0 GHz
    power cap (measured: the whole run slows 1.2x), so every helper matmul
    stays sparse except q's block-diag l2 reduce.
  - v's rstd rides the ACT engine's per-partition scale operand inside the
    PSUM->SBUF copy (v tiles are row-major), via a [P, kcn] rstd column
    obtained from a tiny DRAM-transposed DMA.
  - k is NEVER l2-scaled: keys are partitions in the sim layout, so
    8/||k|| rides the EXP activation's per-partition scale operand
    (exp(sim*scale+bias)), from the same transposed-DMA trick.
  - Helper matmuls (l2 reduce) are issued one m-step late so their
    DVE/ACT input chains never head-of-line-block the PE queue.
  - The PE HAM clock gate only un-throttles to 2.4 GHz under FULL-array
    activity: q is stored twice (each head-half zero-padded to K=128) so the
    sim matmuls contract 128 rows, and v is sliced with a 63-column overhang
    so AV runs M=128 (garbage out-rows 65..127 are never read).
  - Softmax division is deferred and runs per (m, rc) while the next head
    group attends.
"""

import sys

sys.path.insert(0, "/opt/trn_rl_repo")

import numpy as np  # noqa: E402
import ml_dtypes  # noqa: E402

import concourse.bacc as bacc  # noqa: E402
import concourse.bass as bass  # noqa: E402
import concourse.tile as tile  # noqa: E402
from concourse import mybir  # noqa: E402
from concourse.bass_utils import run_bass_kernel_spmd  # noqa: E402

BF = ml_dtypes.bfloat16
F32 = mybir.dt.float32
BF16 = mybir.dt.bfloat16
AF = mybir.ActivationFunctionType
MUL = mybir.AluOpType.mult
COPY = mybir.ActivationFunctionType.Copy

P = 128
N = 2048          # query rows per batch
D = 1024          # model dim
HC = 8            # heads per core
IC = 512          # inner dim per core
NEG = -1.0e4
EPS_LN = 1e-5
SCALE = 8.0

KEYSC = 1152      # key slots: [0:nvis) visible, pads, null at KEYSC-1


def _chunks(total, step=512):
    return [(c, min(c + step, total)) for c in range(0, total, step)]


def build_nc(keysc=KEYSC):
    kcn = keysc // P
    last_kv_ci = (keysc - 1) // 512
    nc = bacc.Bacc(None, target_bir_lowering=False)

    xT_d = nc.dram_tensor("xT", [D, N], BF16, kind="ExternalInput")
    wq_d = nc.dram_tensor("wq", [D, IC], BF16, kind="ExternalInput")
    wk_d = nc.dram_tensor("wk", [D, IC], BF16, kind="ExternalInput")
    wv_d = nc.dram_tensor("wv", [D, IC], BF16, kind="ExternalInput")
    wo_d = nc.dram_tensor("wo", [IC, D], BF16, kind="ExternalInput")
    wsum_d = nc.dram_tensor("wsum", [1, 3 * IC], BF16, kind="ExternalInput")
    nullk_d = nc.dram_tensor("nullk", [P, 4], BF16, kind="ExternalInput")
    nullv_d = nc.dram_tensor("nullv", [1, HC * 65], BF16, kind="ExternalInput")
    mask_d = nc.dram_tensor("maskcol", [P, kcn], F32, kind="ExternalInput")
    qks2_d = nc.dram_tensor("qks2", [P, 1], F32, kind="ExternalInput")
    out_d = nc.dram_tensor("out", [N, D], F32, kind="ExternalOutput")

    with tile.TileContext(nc) as tc:
        with (
            tc.tile_pool(name="consts", bufs=1) as cns,
            tc.tile_pool(name="qkv", bufs=1) as qkv,
            tc.tile_pool(name="wop", bufs=1) as wop,
            tc.tile_pool(name="rep", bufs=4) as repp,
            tc.tile_pool(name="dram", bufs=1, space="DRAM") as drp,
        ):
            qTh = [qkv.tile([P, 4, N], BF16, name=f"qTh{i}")
                   for i in range(2)]
            kT = qkv.tile([P, 4, keysc], BF16)
            v_sb = qkv.tile([P, kcn, HC * 65 + 63], BF16)
            oT = qkv.tile([P, 4, N], BF16)

            den_dr = drp.tile([8, N], BF16, name="den")
            rec_dr = drp.tile([8, N], BF16, name="rec")
            sd_dr = drp.tile([1, keysc], F32, name="sd")
            kn_dr = drp.tile([8, keysc], BF16, name="kn")

            # ---------- phase A+B: LN, projections, l2 norms ----------
            with (
                tc.tile_pool(name="xp", bufs=1) as xp,
                tc.tile_pool(name="wp", bufs=1) as wp,
                tc.tile_pool(name="asml", bufs=2) as sml,
                tc.tile_pool(name="ascr", bufs=2) as scr,
                tc.tile_pool(name="lnps", bufs=2, space="PSUM") as lnps,
                tc.tile_pool(name="pjps", bufs=4, space="PSUM") as pjps,
            ):
                xre = xT_d.rearrange("(f p) r -> f p r", p=P)
                # first x chunk ahead of everything else in the DMA queues
                xc0 = xp.tile([P, 8, 512], BF16, tag="xc", bufs=2, name="xc")
                for f in range(8):
                    nc.sync.dma_start(out=xc0[:, f, :],
                                      in_=xre[f, :, 0:512])
                wq_sb = wp.tile([P, 8, IC], BF16, tag="wq")
                wk_sb = wp.tile([P, 8, IC], BF16, tag="wk")
                wv_sb = wp.tile([P, 8, IC], BF16, tag="wv")
                nc.sync.dma_start(
                    out=wq_sb, in_=wq_d.rearrange("(f p) j -> p f j", p=P))
                nc.sync.dma_start(
                    out=wk_sb, in_=wk_d.rearrange("(f p) j -> p f j", p=P))
                nc.sync.dma_start(
                    out=wv_sb, in_=wv_d.rearrange("(f p) j -> p f j", p=P))

                ones1b = cns.tile([P, 1], BF16)
                nc.vector.memset(ones1b, 1.0)
                blkd128 = cns.tile([P, P], BF16)
                nc.vector.memset(blkd128, 0.0)
                nc.vector.memset(blkd128[0:64, 0:64], 1.0)
                nc.vector.memset(blkd128[64:128, 64:128], 1.0)
                blkdiag = cns.tile([P, 2], BF16)
                nc.vector.memset(blkdiag, 0.0)
                nc.vector.memset(blkdiag[0:64, 0:1], 1.0)
                nc.vector.memset(blkdiag[64:128, 1:2], 1.0)
                wsum_sb = cns.tile([1, 3 * IC], BF16)
                nc.sync.dma_start(out=wsum_sb, in_=wsum_d[:, :])
                maskc = cns.tile([P, kcn], F32)
                nc.sync.dma_start(out=maskc, in_=mask_d[:, :])
                nullk_sb = cns.tile([P, 4], BF16)
                nc.sync.dma_start(out=nullk_sb, in_=nullk_d[:, :])
                qks2_sb = cns.tile([P, 1], F32)
                nc.sync.dma_start(out=qks2_sb, in_=qks2_d[:, :])
                eps_col = cns.tile([P, 1], F32)
                nc.vector.memset(eps_col, EPS_LN)
                s_colT = cns.tile([P, kcn], F32)
                wo_sb = wop.tile([P, 4, D], BF16)
                nc.sync.dma_start(
                    out=wo_sb, in_=wo_d.rearrange("(m p) j -> p m j", p=P))
                nc.vector.memset(
                    v_sb[:, :, 0:HC * 65].rearrange(
                        "p t (h c) -> p t h c", c=65)[:, :, :, 64:65],
                    1.0)
                nc.vector.memset(v_sb[:, :, HC * 65:], 0.0)
                nc.vector.memset(qTh[0][64:128, :, :], 0.0)
                nc.vector.memset(qTh[1][0:64, :, :], 0.0)

                def q_l2_chain(m, c0, c1, n2):
                    # n2 already matmul'd: [P, w] PSUM broadcast of ||q||^2
                    w = c1 - c0
                    r2 = sml.tile([P, 512], F32, tag="r2", name="r2")
                    nc.vector.reciprocal_approx_fast(r2[:, 0:w], n2[:, 0:w])
                    nb = sml.tile([P, 512], BF16, tag="nb", name="nb")
                    nc.scalar.activation(nb[:, 0:w], r2[:, 0:w], AF.Sqrt,
                                         scale=qks2_sb[:, 0:1])
                    nc.vector.tensor_mul(qTh[0][0:64, m, c0:c1],
                                         qTh[0][0:64, m, c0:c1],
                                         nb[0:64, 0:w])
                    nc.vector.tensor_mul(qTh[1][64:128, m, c0:c1],
                                         qTh[1][64:128, m, c0:c1],
                                         nb[64:128, 0:w])

                def k_l2_chain(m, c0, k1, n2k):
                    # n2k: [2, kw] PSUM ||k||^2 rows -> 8/||k|| folded into
                    # kT via a per-half DMA broadcast (keeps the attention
                    # EXP bias-only: a scale AP costs ~90ns per EXP on the
                    # bottleneck ACT engine)
                    kw = k1 - c0
                    r2k = sml.tile([2, 512], F32, tag="r2k", name="r2k")
                    nc.vector.reciprocal_approx_fast(r2k[:, 0:kw],
                                                     n2k[:, 0:kw])
                    knr = sml.tile([2, 512], BF16, tag="knr", name="knr")
                    nc.scalar.activation(knr[:, 0:kw], r2k[:, 0:kw], AF.Sqrt,
                                         scale=float(SCALE * SCALE))
                    nc.sync.dma_start(out=kn_dr[2 * m:2 * m + 2, c0:k1],
                                      in_=knr[:, 0:kw])
                    repk = repp.tile([P, 512], BF16, tag="rpk", bufs=2,
                                     name="repk")
                    for h2 in range(2):
                        src = kn_dr[2 * m + h2, c0:k1]
                        nc.sync.dma_start(
                            out=repk[64 * h2:64 * (h2 + 1), 0:kw],
                            in_=bass.AP(tensor=src.tensor, offset=src.offset,
                                        ap=[[0, 64]] + src.ap))
                    nc.vector.tensor_mul(kT[:, m, c0:k1], kT[:, m, c0:k1],
                                         repk[:, 0:kw])

                for ci, (c0, c1) in enumerate(_chunks(N)):
                    w = c1 - c0
                    if ci > 0:
                        xc0 = xp.tile([P, 8, 512], BF16, tag="xc", bufs=2,
                                      name="xc")
                        for f in range(8):
                            nc.sync.dma_start(out=xc0[:, f, :],
                                              in_=xre[f, :, c0:c1])
                    k1 = min(c1, keysc)
                    kvw = k1 - c0 if c0 < keysc else 0
                    # row sums (M=1, low power) + raw-x square sums; the
                    # square-sum matmuls lag 2 steps behind their DVE squares
                    sA = lnps.tile([1, 512], F32, tag="lA", name="sA")
                    sB = None
                    sqs = {}
                    for f in range(8):
                        nc.tensor.matmul(sA[:, 0:w], ones1b, xc0[:, f, 0:w],
                                         start=(f == 0), stop=(f == 7))
                        if kvw > 0:
                            sq = scr.tile([P, 512], BF16, tag="sq", bufs=4,
                                          name="sq")
                            nc.vector.tensor_mul(sq[:, 0:kvw],
                                                 xc0[:, f, 0:kvw],
                                                 xc0[:, f, 0:kvw])
                            sqs[f] = sq
                            if f >= 2:
                                if sB is None:
                                    sB = lnps.tile([1, 512], F32, tag="lB",
                                                   name="sB")
                                nc.tensor.matmul(sB[:, 0:kvw], ones1b,
                                                 sqs.pop(f - 2)[:, 0:kvw],
                                                 start=(f == 2), stop=False)
                    mu_row = sml.tile([1, 512], BF16, tag="mu", name="mu_row")
                    nc.scalar.activation(mu_row[:, 0:w], sA[:, 0:w], COPY,
                                         0.0, 1.0 / float(D))
                    if kvw > 0:
                        for f in (6, 7):
                            nc.tensor.matmul(sB[:, 0:kvw], ones1b,
                                             sqs.pop(f)[:, 0:kvw],
                                             start=False, stop=(f == 7))
                        # sd = sqrt(E[x^2] - mu^2 + eps) row; rstd lands
                        # transposed in s_colT for the v-copy scale operand
                        a1 = sml.tile([1, 512], F32, tag="a1", name="a1")
                        nc.scalar.activation(a1[:, 0:kvw], sA[:, 0:kvw],
                                             AF.Square, 0.0, 1.0 / float(D))
                        t1 = sml.tile([1, 512], F32, tag="t1", name="t1")
                        nc.vector.tensor_scalar(t1[:, 0:kvw], sB[:, 0:kvw],
                                                1.0 / float(D), None, MUL)
                        nc.vector.tensor_sub(t1[:, 0:kvw], t1[:, 0:kvw],
                                             a1[:, 0:kvw])
                        sd_b = sml.tile([1, 512], F32, tag="sd", name="sd_b")
                        nc.scalar.activation(sd_b[:, 0:kvw], t1[:, 0:kvw],
                                             AF.Sqrt, bias=eps_col[0:1, 0:1])
                        nc.sync.dma_start(out=sd_dr[0:1, c0:k1],
                                          in_=sd_b[0:1, 0:kvw])
                        nt = kvw // P
                        sdT = sml.tile([P, 4], F32, tag="sdT", name="sdT")
                        src = sd_dr[0, c0:k1]
                        nc.sync.dma_start(
                            out=sdT[:, 0:nt],
                            in_=bass.AP(tensor=src.tensor, offset=src.offset,
                                        ap=[[1, P], [P, nt]]))
                        nc.vector.reciprocal_approx_fast(
                            s_colT[:, c0 // P:c0 // P + nt], sdT[:, 0:nt])
                    # q projection; l2 helper matmul deferred one m-step so
                    # its DVE/ACT inputs never stall the PE queue
                    pend = None
                    for m in range(4):
                        qp = pjps.tile([P, 512], F32, tag="pj", name="qp")
                        for f in range(8):
                            nc.tensor.matmul(
                                qp[:, 0:w], wq_sb[:, f, m * P:(m + 1) * P],
                                xc0[:, f, 0:w],
                                start=(f == 0), stop=False)
                        nc.tensor.matmul(qp[:, 0:w],
                                         wsum_sb[0:1, m * P:(m + 1) * P],
                                         mu_row[:, 0:w],
                                         start=False, stop=True)
                        if pend is not None:
                            pm, psq = pend
                            n2 = pjps.tile([P, 512], F32, tag="pj", name="n2")
                            nc.tensor.matmul(n2[:, 0:w], blkd128,
                                             psq[:, 0:w],
                                             start=True, stop=True)
                            q_l2_chain(pm, c0, c1, n2)
                        nc.scalar.copy(qTh[0][0:64, m, c0:c1],
                                       qp[0:64, 0:w])
                        nc.vector.tensor_copy(qTh[1][64:128, m, c0:c1],
                                              qp[64:128, 0:w])
                        sq2 = scr.tile([P, 512], BF16, tag="sq2", name="sq2")
                        nc.vector.tensor_mul(sq2[0:64, 0:w],
                                             qTh[0][0:64, m, c0:c1],
                                             qTh[0][0:64, m, c0:c1])
                        nc.vector.tensor_mul(sq2[64:128, 0:w],
                                             qTh[1][64:128, m, c0:c1],
                                             qTh[1][64:128, m, c0:c1])
                        pend = (m, sq2)
                    if kvw > 0:
                        # k projection; transposed-norm helper also deferred
                        kpend = pend
                        for m in range(4):
                            kp = pjps.tile([P, 512], F32, tag="pj", name="kp")
                            for f in range(8):
                                nc.tensor.matmul(
                                    kp[:, 0:kvw],
                                    wk_sb[:, f, m * P:(m + 1) * P],
                                    xc0[:, f, 0:kvw],
                                    start=(f == 0), stop=False)
                            nc.tensor.matmul(
                                kp[:, 0:kvw],
                                wsum_sb[0:1, IC + m * P:IC + (m + 1) * P],
                                mu_row[:, 0:kvw],
                                start=False, stop=True)
                            if kpend is not None and kpend[0] == 3 and \
                                    len(kpend) == 2:
                                pm, psq = kpend
                                n2 = pjps.tile([P, 512], F32, tag="pj",
                                               name="n2")
                                nc.tensor.matmul(n2[:, 0:w], blkd128,
                                                 psq[:, 0:w],
                                                 start=True, stop=True)
                                q_l2_chain(pm, c0, c1, n2)
                            elif kpend is not None:
                                pm, psq, pk1 = kpend
                                n2k = pjps.tile([2, 512], F32, tag="pj",
                                                name="n2k")
                                nc.tensor.matmul(n2k[:, 0:k1 - c0], blkdiag,
                                                 psq[:, 0:k1 - c0],
                                                 start=True, stop=True)
                                k_l2_chain(pm, c0, pk1, n2k)
                            nc.scalar.copy(kT[:, m, c0:k1], kp[:, 0:kvw])
                            if ci == last_kv_ci:
                                # k projected for this m: null k overwrite
                                # must precede this m's norm squares
                                nc.sync.dma_start(
                                    out=kT[:, m, keysc - 1:keysc],
                                    in_=nullk_sb[:, m:m + 1])
                            sqk = scr.tile([P, 512], BF16, tag="sq2",
                                           name="sqk")
                            nc.vector.tensor_mul(sqk[:, 0:kvw],
                                                 kT[:, m, c0:k1],
                                                 kT[:, m, c0:k1])
                            kpend = (m, sqk, k1)
                        # v projection: rstd applied inside the PSUM->SBUF
                        # copy via ACT per-partition scale
                        for rt in range(c0 // P, k1 // P):
                            rl = (rt - c0 // P) * P
                            vp = pjps.tile([P, 512], F32, tag="pj", name="vp")
                            for f in range(8):
                                nc.tensor.matmul(
                                    vp, xc0[:, f, rl:rl + P],
                                    wv_sb[:, f, :],
                                    start=(f == 0), stop=False)
                            nc.tensor.matmul(
                                vp, mu_row[0:1, rl:rl + P],
                                wsum_sb[0:1, 2 * IC:3 * IC],
                                start=False, stop=True)
                            if kpend is not None:
                                pm, psq, pk1 = kpend
                                n2k = pjps.tile([2, 512], F32, tag="pj",
                                                name="n2k")
                                nc.tensor.matmul(n2k[:, 0:pk1 - c0], blkdiag,
                                                 psq[:, 0:pk1 - c0],
                                                 start=True, stop=True)
                                k_l2_chain(pm, c0, pk1, n2k)
                                kpend = None
                            nc.scalar.activation(
                                v_sb[:, rt, 0:HC * 65].rearrange(
                                    "p (h c) -> p h c", c=65)[:, :, 0:64],
                                vp.rearrange("p (h c) -> p h c", c=64),
                                COPY, 0.0, s_colT[:, rt:rt + 1])
                        if ci == last_kv_ci:
                            nc.sync.dma_start(
                                out=v_sb[127:128, kcn - 1, 0:HC * 65],
                                in_=nullv_d[:, :])
                    else:
                        # last chunk: flush the final q l2 helper
                        pm, psq = pend
                        n2 = pjps.tile([P, 512], F32, tag="pj", name="n2")
                        nc.tensor.matmul(n2[:, 0:w], blkd128, psq[:, 0:w],
                                         start=True, stop=True)
                        q_l2_chain(pm, c0, c1, n2)

            # ---------- phase C: attention + per-m softmax division --------
            with (
                tc.tile_pool(name="accp", bufs=2, space="PSUM") as accp,
                tc.tile_pool(name="simp", bufs=2, space="PSUM") as simp,
                tc.tile_pool(name="expp", bufs=3) as expp,
                tc.tile_pool(name="omp", bufs=3) as omp,
                tc.tile_pool(name="dsml", bufs=2) as dsml,
            ):
                for m in range(4):
                    for rc in range(2):
                        ops = [accp.tile([P, 1024], F32, tag="acc",
                                         name=f"av{i}") for i in range(2)]
                        for kc in range(kcn):
                            for h2 in range(2):
                                sim = simp.tile([P, 1024], F32, tag="sim",
                                                name="sim")
                                for nh in range(2):
                                    r0 = rc * 1024 + nh * 512
                                    nc.tensor.matmul(
                                        sim[:, nh * 512:(nh + 1) * 512],
                                        kT[:, m, kc * P:(kc + 1) * P],
                                        qTh[h2][:, m, r0:r0 + 512],
                                        start=True, stop=True)
                                e = expp.tile([P, 1024], BF16, tag="e",
                                              name="e")
                                nc.scalar.activation(
                                    e, sim, AF.Exp,
                                    bias=maskc[:, kc:kc + 1])
                                for nh in range(2):
                                    nc.tensor.matmul(
                                        ops[h2][:, nh * 512:(nh + 1) * 512],
                                        v_sb[:, kc,
                                             (2 * m + h2) * 65:
                                             (2 * m + h2) * 65 + 128],
                                        e[:, nh * 512:(nh + 1) * 512],
                                        start=(kc == 0), stop=(kc == kcn - 1))
                        for h2 in range(2):
                            om = omp.tile([65, 1024], BF16, tag="om",
                                          name="om")
                            with nc.allow_low_precision("bf16 numer/denom"):
                                nc.vector.tensor_copy(om, ops[h2][0:65, :])
                            nc.sync.dma_start(
                                out=oT[64 * h2:64 * (h2 + 1), m,
                                       rc * 1024:(rc + 1) * 1024],
                                in_=om[0:64, :])
                            nc.sync.dma_start(
                                out=den_dr[2 * m + h2:2 * m + h2 + 1,
                                           rc * 1024:(rc + 1) * 1024],
                                in_=om[64:65, :])
                        # divide this rc-half while attention continues
                        q0 = rc * 1024
                        dsb = dsml.tile([2, 1024], BF16, tag="dsb",
                                        name="dsb")
                        nc.sync.dma_start(
                            out=dsb,
                            in_=den_dr[2 * m:2 * m + 2, q0:q0 + 1024])
                        dff = dsml.tile([2, 1024], F32, tag="dff", name="dff")
                        nc.vector.tensor_copy(dff, dsb)
                        drf = dsml.tile([2, 1024], F32, tag="dff", name="drf")
                        nc.vector.reciprocal_approx_fast(drf, dff)
                        drb = dsml.tile([2, 1024], BF16, tag="dsb",
                                        name="drb")
                        with nc.allow_low_precision("bf16 recip"):
                            nc.vector.tensor_copy(drb, drf)
                        nc.sync.dma_start(
                            out=rec_dr[2 * m:2 * m + 2, q0:q0 + 1024],
                            in_=drb)
                        repd = repp.tile([P, 1024], BF16, tag="rpd", bufs=2,
                                         name="repd")
                        for h2 in range(2):
                            src = rec_dr[2 * m + h2, q0:q0 + 1024]
                            nc.sync.dma_start(
                                out=repd[64 * h2:64 * (h2 + 1), :],
                                in_=bass.AP(tensor=src.tensor,
                                            offset=src.offset,
                                            ap=[[0, 64]] + src.ap))
                        nc.vector.tensor_mul(oT[:, m, q0:q0 + 1024],
                                             oT[:, m, q0:q0 + 1024], repd)

            # ---------- phase D: output projection ----------
            with (
                tc.tile_pool(name="dps", bufs=4, space="PSUM") as dps,
                tc.tile_pool(name="dscr", bufs=3) as scr2,
            ):
                for rt in range(16):
                    for n2 in range(2):
                        op = dps.tile([P, 512], F32, tag="op", name="op")
                        for m in range(4):
                            nc.tensor.matmul(
                                op, oT[:, m, rt * P:(rt + 1) * P],
                                wo_sb[:, m, n2 * 512:(n2 + 1) * 512],
                                start=(m == 0), stop=(m == 3))
                        sg = scr2.tile([P, 512], F32, tag="sg", name="sg")
                        if n2 == 0:
                            nc.scalar.copy(sg, op)
                        else:
                            nc.vector.tensor_copy(sg, op)
                        nc.sync.dma_start(
                            out=out_d[rt * P:(rt + 1) * P,
                                      n2 * 512:(n2 + 1) * 512],
                            in_=sg)

    nc.finalize()
    return nc


_NC = {}


def _get_nc(keysc=KEYSC):
    if keysc not in _NC:
        _NC[keysc] = build_nc(keysc)
    return _NC[keysc]


def _shards(x, context_mask, gamma, wq, wkv, null_kv, q_scale, k_scale, wo,
            keysc):
    kcn = keysc // P
    x = np.asarray(x, np.float32)
    gamma = np.asarray(gamma, np.float32)
    wq_g = np.asarray(wq, np.float32) * gamma[:, None]
    wkv_g = np.asarray(wkv, np.float32) * gamma[:, None]
    wk_g = wkv_g[:, :D]
    wv_g = wkv_g[:, D:]
    wo = np.asarray(wo, np.float32)
    null_kv = np.asarray(null_kv, np.float32)
    cm = np.asarray(context_mask)
    qs = np.asarray(q_scale, np.float32)
    ks = np.asarray(k_scale, np.float32)
    qks2 = (np.tile(qs * ks, 2) ** 2).astype(np.float32)[:, None]  # [128,1]

    maps, perms = [], []
    for c in range(8):
        b, hh = c // 2, c % 2
        sl = slice(hh * IC, (hh + 1) * IC)
        heads = np.arange(HC) + hh * HC
        nk = null_kv[0][heads, 0, :]
        nv = null_kv[1][heads, 0, :]
        nullk = np.ascontiguousarray(
            nk.reshape(4, 2, 64).transpose(1, 2, 0).reshape(P, 4))
        nullv = np.zeros((1, HC * 65), np.float32)
        for h in range(HC):
            nullv[0, h * 65:h * 65 + 64] = nv[h]
            nullv[0, h * 65 + 64] = 1.0
        wsum = np.concatenate([
            -wq_g[:, sl].sum(axis=0),
            -wk_g[:, sl].sum(axis=0),
            -wv_g[:, sl].sum(axis=0)])[None, :]
        vis = np.flatnonzero(cm[b])
        perm = np.concatenate([vis, np.flatnonzero(~cm[b])])
        perms.append(perm)
        nvis = len(vis)
        bias = np.zeros((keysc,), np.float32)
        bias[nvis:] = NEG
        bias[keysc - 1] = 0.0        # null key always visible
        maskcol = np.ascontiguousarray(bias.reshape(kcn, P).T)
        xp = x[b][perm]
        maps.append({
            "xT": np.ascontiguousarray(xp.T).astype(BF),
            "wq": np.ascontiguousarray(wq_g[:, sl]).astype(BF),
            "wk": np.ascontiguousarray(wk_g[:, sl]).astype(BF),
            "wv": np.ascontiguousarray(wv_g[:, sl]).astype(BF),
            "wo": np.ascontiguousarray(wo[sl, :]).astype(BF),
            "wsum": np.ascontiguousarray(wsum).astype(BF),
            "nullk": nullk.astype(BF),
            "nullv": nullv.astype(BF),
            "maskcol": maskcol,
            "qks2": qks2,
        })
    return maps, perms


def kernel(x, context_mask, gamma, wq, wkv, null_kv, q_scale, k_scale, wo,
           _trace=False):
    cm = np.asarray(context_mask)
    max_vis = int(cm.sum(axis=1).max())
    # need max_vis visible slots plus the null key at slot keysc-1
    keysc = max(KEYSC, ((max_vis + 1 + P - 1) // P) * P)
    nc = _get_nc(keysc)
    maps, perms = _shards(x, context_mask, gamma, wq, wkv, null_kv,
                          q_scale, k_scale, wo, keysc)
    res = run_bass_kernel_spmd(nc, maps, core_ids=list(range(8)),
                               trace=_trace,
                               tmpdir="/tmp/bass_trace" if _trace else None)
    outs = [np.asarray(res.results[c]["out"], np.float32) for c in range(8)]
    full = np.empty((4, N, D), np.float32)
    for b in range(4):
        full[b, perms[2 * b], :] = outs[2 * b] + outs[2 * b + 1]
    if _trace:
        kernel.last_exec_time_ns = res.exec_time_ns
    return full
